# revision 2
# baseline (speedup 1.0000x reference)
"""Trainium2 Bass kernel for nn_GemNetOutput (segment_reduce + FiLM + MLP head).

Reference computation (all fp32):
    g     = segment_sum(x, batch, num_segments=B)        # [B, H]
    gamma = domain_emb @ gamma_w.T + gamma_b             # [B, H]
    beta  = domain_emb @ beta_w.T  + beta_b              # [B, H]
    g     = gamma * g + beta
    h     = silu(g @ w1.T + b1)                          # [B, H]
    h     = silu(h @ w2.T + b2)                          # [B, H/2]
    out   = (h @ w3.T + b3).squeeze(-1)                  # [B]

Shapes: N=1e6 nodes, B=16384 graphs, H=512, FD=16.  `batch` is SORTED.

Strategy (8 NeuronCores, no collectives needed):
  - Shard by SEGMENT range: core c owns segments [c*2048, (c+1)*2048), i.e.
    one contiguous node slice of x (batch is sorted).  16 windows of 128
    segments per core.
  - x is quantized to fp8 E3M4 on the host with per-(segment, feature)
    ERROR DIFFUSION: q_i = rne_e3m4(x_i + carry), carry += x_i - q_i.  The
    segment sum of the quantized stream then telescopes to the true sum
    minus one final carry (<= ULP/2), so the fp8 rounding does NOT
    accumulate sqrt(n)-style.  Odd-length segments get one pad slot that
    absorbs the final carry.  Measured end-to-end rel err ~4e-3, same as a
    bf16-x scheme, at HALF the HBM traffic (512 MB total).
  - Each window's (padded) nodes are packed into PAIRS laid out as
    [p=128, pair-half=2, col, H].  The DVE adds the two halves (fp8 ->
    bf16, exact to ~bf16) which halves the PE one-hot matmul count.
  - segment_sum on the PE: per pair-column, build a one-hot [pair, seg]
    matrix (tensor_scalar is_equal vs an iota row) and accumulate matmuls
    into a PSUM [128 seg, 512] tile.
  - x DMAs are partition-contiguous: one 2 MB transfer per half-window
    (16 KB contiguous per partition) for near-peak HBM bandwidth.
  - FiLM + MLP run per-window on-device in transposed [feature, seg]
    layout (PE transpose), biases folded via per-partition activation
    bias.  MLP weights/activations bf16, accumulation fp32.
"""

import sys
from contextlib import ExitStack

for _p in ("/opt/trn_rl_repo", "/opt/pypackages"):
    if _p not in sys.path:
        sys.path.append(_p)

import ml_dtypes
import numpy as np

import concourse.bass as bass
import concourse.tile as tile
from concourse import bacc, mybir
from concourse import bass_utils

dt = mybir.dt

# Problem constants (hardcoded per the contract).
N_NODES = 1_000_000
B_SEGS = 16_384
H = 512
H2 = 256
FD = 16
N_CORES = 8
SEG_W = 128          # segments per window (PSUM partition dim)
WINDOWS = (B_SEGS // N_CORES) // SEG_W   # 16

BF16 = ml_dtypes.bfloat16
E3M4 = ml_dtypes.float8_e3m4
E3M4_MAX = 15.5

# CoreSim has no Silu LUT; compose silu = z * sigmoid(z) when True (sim tests).
SILU_COMPOSE = False


def build_program(cp2: int, n_cores: int):
    """Build the per-core Bass/Tile program.

    cp2: pair-columns per half-window DMA (Cp = 2*cp2 pair columns per
    window; window capacity = 128 * Cp pairs = 256 * Cp node slots).
    """
    spc = WINDOWS * SEG_W
    m_dt = dt.bfloat16

    nc = bacc.Bacc(
        "TRN2",
        target_bir_lowering=False,
        debug=False,
        enable_asserts=False,
        num_devices=n_cores,
    )

    xp = nc.dram_tensor(
        "xp", [WINDOWS, 2, 128, 2, cp2, H], dt.float8e3, kind="ExternalInput").ap()
    brtA = nc.dram_tensor(
        "brtA", [128, WINDOWS, 2, cp2], dt.float32, kind="ExternalInput").ap()
    dombT = nc.dram_tensor("dombT", [FD + 1, spc], dt.float32, kind="ExternalInput").ap()
    gw = nc.dram_tensor("gw", [FD + 1, H], dt.float32, kind="ExternalInput").ap()
    bw = nc.dram_tensor("bw", [FD + 1, H], dt.float32, kind="ExternalInput").ap()
    w1t = nc.dram_tensor("w1t", [H, H], m_dt, kind="ExternalInput").ap()
    w2t = nc.dram_tensor("w2t", [H, H2], m_dt, kind="ExternalInput").ap()
    w3c = nc.dram_tensor("w3c", [128, H2 // 128], m_dt, kind="ExternalInput").ap()
    b1c = nc.dram_tensor("b1c", [128, H // 128], dt.float32, kind="ExternalInput").ap()
    b2c = nc.dram_tensor("b2c", [128, H2 // 128], dt.float32, kind="ExternalInput").ap()
    b3c = nc.dram_tensor("b3c", [1, 1], dt.float32, kind="ExternalInput").ap()
    iden = nc.dram_tensor("iden", [128, 128], dt.float32, kind="ExternalInput").ap()
    iotr = nc.dram_tensor("iotr", [128, 128], m_dt, kind="ExternalInput").ap()
    out = nc.dram_tensor("out", [1, spc], dt.float32, kind="ExternalOutput").ap()

    HC = H // 128       # 4 h-chunks
    JC = H // 128       # 4 layer-1 output chunks
    KC = H2 // 128      # 2 layer-2 output chunks

    with tile.TileContext(nc) as tc, ExitStack() as ctx:
        cpool = ctx.enter_context(tc.tile_pool(name="consts", bufs=1))
        xpool = ctx.enter_context(tc.tile_pool(name="x", bufs=4))
        xspool = ctx.enter_context(tc.tile_pool(name="xs", bufs=4))
        ohpool = ctx.enter_context(tc.tile_pool(name="oh", bufs=8))
        spool = ctx.enter_context(tc.tile_pool(name="work", bufs=2))
        pg = ctx.enter_context(tc.tile_pool(name="pg", bufs=3, space=bass.MemorySpace.PSUM))
        pt = ctx.enter_context(tc.tile_pool(name="pt", bufs=2, space=bass.MemorySpace.PSUM))
        pm = ctx.enter_context(tc.tile_pool(name="pm", bufs=2, space=bass.MemorySpace.PSUM))

        # ---- constants / weights into SBUF ----
        iden_sb = cpool.tile([128, 128], dt.float32)
        nc.sync.dma_start(iden_sb[:], iden)
        iotr_sb = cpool.tile([128, 128], m_dt)
        nc.sync.dma_start(iotr_sb[:], iotr)
        w1_sb = cpool.tile([128, HC, H], m_dt)
        nc.sync.dma_start(w1_sb[:], w1t.rearrange("(c p) j -> p c j", p=128))
        w2_sb = cpool.tile([128, HC, H2], m_dt)
        nc.sync.dma_start(w2_sb[:], w2t.rearrange("(c p) j -> p c j", p=128))
        w3_sb = cpool.tile([128, KC], m_dt)
        nc.sync.dma_start(w3_sb[:], w3c)
        b1_sb = cpool.tile([128, JC], dt.float32)
        nc.sync.dma_start(b1_sb[:], b1c)
        b2_sb = cpool.tile([128, KC], dt.float32)
        nc.sync.dma_start(b2_sb[:], b2c)
        b3_sb = cpool.tile([1, 1], dt.float32)
        nc.sync.dma_start(b3_sb[:], b3c)
        gw_sb = cpool.tile([FD + 1, H], dt.float32)
        nc.sync.dma_start(gw_sb[:], gw)
        bw_sb = cpool.tile([FD + 1, H], dt.float32)
        nc.sync.dma_start(bw_sb[:], bw)
        domT_sb = cpool.tile([FD + 1, spc], dt.float32)
        nc.sync.dma_start(domT_sb[:], dombT)
        brt_sb = cpool.tile([128, WINDOWS, 2, cp2], dt.float32)
        nc.sync.dma_start(brt_sb[:], brtA)
        out_sb = cpool.tile([1, spc], dt.float32)

        is_eq = mybir.AluOpType.is_equal

        # ---- PE warm-up: ~5us of dummy matmuls while DMA prefills, so HAM
        # flips to K=8/8 before the real stream starts.
        warm_t = pm.tile([128, H], dt.float32, tag="pmlp")
        for i in range(48):
            nc.tensor.matmul(
                warm_t[:, 0:128], iotr_sb[:], iotr_sb[:],
                start=(i == 0), stop=(i == 47))

        for w in range(WINDOWS):
            # --- gamma/beta for this window: [128 h-part, HC, SEG_W] ---
            g_sbt = spool.tile([128, HC, SEG_W], dt.float32, tag="gbg_g")
            b_sbt = spool.tile([128, HC, SEG_W], dt.float32, tag="gbg_b")
            dom_s = domT_sb[:, w * SEG_W:(w + 1) * SEG_W]
            for hc in range(HC):
                for wsb, dst in ((gw_sb, g_sbt), (bw_sb, b_sbt)):
                    pgb_t = pm.tile([128, H], dt.float32, tag="pmlp")
                    nc.tensor.matmul(
                        pgb_t[:, 0:SEG_W],
                        wsb[:, hc * 128:(hc + 1) * 128], dom_s,
                        start=True, stop=True)
                    nc.scalar.copy(dst[:, hc, :], pgb_t[:, 0:SEG_W])

            # --- segment-sum for this window: accumulate [128 seg, H] ---
            pg_t = pg.tile([128, H], dt.float32)
            for jh in range(2):
                xt = xpool.tile([128, 2, cp2, H], dt.float8e3)
                nc.sync.dma_start(xt[:], xp[w, jh])
                xs = xspool.tile([128, cp2, H], m_dt)
                nc.vector.tensor_add(xs[:], xt[:, 0], xt[:, 1])
                for j in range(cp2):
                    oh = ohpool.tile([128, 128], m_dt)
                    nc.vector.tensor_scalar(
                        oh[:], iotr_sb[:], brt_sb[:, w, jh, j:j + 1], None, is_eq)
                    nc.tensor.matmul(
                        pg_t[:], oh[:], xs[:, j, :],
                        start=(jh == 0 and j == 0),
                        stop=(jh == 1 and j == cp2 - 1))

            # --- evict g to SBUF, transpose, apply FiLM ---
            g_sb = spool.tile([128, H], dt.float32, tag="g")
            nc.scalar.copy(g_sb[:], pg_t[:])
            pt_t = pt.tile([128, H], dt.float32)
            for hc in range(HC):
                nc.tensor.transpose(
                    pt_t[:, hc * 128:(hc + 1) * 128],
                    g_sb[:, hc * 128:(hc + 1) * 128],
                    iden_sb[:])
            gmodT = spool.tile([128, H], m_dt, tag="gmodT")
            pt_v = pt_t[:].rearrange("p (c s) -> p c s", c=HC)
            gm_v = gmodT[:].rearrange("p (c s) -> p c s", c=HC)
            nc.vector.tensor_mul(gm_v, pt_v, g_sbt[:, :, :])
            nc.vector.tensor_add(gm_v, gm_v, b_sbt[:, :, :])

            # --- MLP layer 1: h1T[j, s] = silu(sum_h w1t[h, j] gmodT[h, s] + b1[j]) ---
            ph1 = pm.tile([128, H], dt.float32, tag="pmlp")
            for jc in range(JC):
                for hc in range(HC):
                    nc.tensor.matmul(
                        ph1[:, jc * 128:(jc + 1) * 128],
                        w1_sb[:, hc, jc * 128:(jc + 1) * 128],
                        gmodT[:, hc * 128:(hc + 1) * 128],
                        start=(hc == 0), stop=(hc == HC - 1))
            h1_sb = spool.tile([128, H], m_dt, tag="h1")
            if SILU_COMPOSE:
                z1 = spool.tile([128, H], dt.float32, tag="z1")
                for jc in range(JC):
                    nc.scalar.activation(
                        z1[:, jc * 128:(jc + 1) * 128],
                        ph1[:, jc * 128:(jc + 1) * 128],
                        mybir.ActivationFunctionType.Identity,
                        bias=b1_sb[:, jc:jc + 1])
                nc.scalar.activation(
                    h1_sb[:], z1[:], mybir.ActivationFunctionType.Sigmoid)
                nc.vector.tensor_mul(h1_sb[:], h1_sb[:], z1[:])
            else:
                for jc in range(JC):
                    nc.scalar.activation(
                        h1_sb[:, jc * 128:(jc + 1) * 128],
                        ph1[:, jc * 128:(jc + 1) * 128],
                        mybir.ActivationFunctionType.Silu,
                        bias=b1_sb[:, jc:jc + 1])

            # --- MLP layer 2 ---
            ph2 = pm.tile([128, H2], dt.float32, tag="pmlp")
            for kc in range(KC):
                for hc in range(HC):
                    nc.tensor.matmul(
                        ph2[:, kc * 128:(kc + 1) * 128],
                        w2_sb[:, hc, kc * 128:(kc + 1) * 128],
                        h1_sb[:, hc * 128:(hc + 1) * 128],
                        start=(hc == 0), stop=(hc == HC - 1))
            h2_sb = spool.tile([128, H2], m_dt, tag="h2")
            if SILU_COMPOSE:
                z2 = spool.tile([128, H2], dt.float32, tag="z2")
                for kc in range(KC):
                    nc.scalar.activation(
                        z2[:, kc * 128:(kc + 1) * 128],
                        ph2[:, kc * 128:(kc + 1) * 128],
                        mybir.ActivationFunctionType.Identity,
                        bias=b2_sb[:, kc:kc + 1])
                nc.scalar.activation(
                    h2_sb[:], z2[:], mybir.ActivationFunctionType.Sigmoid)
                nc.vector.tensor_mul(h2_sb[:], h2_sb[:], z2[:])
            else:
                for kc in range(KC):
                    nc.scalar.activation(
                        h2_sb[:, kc * 128:(kc + 1) * 128],
                        ph2[:, kc * 128:(kc + 1) * 128],
                        mybir.ActivationFunctionType.Silu,
                        bias=b2_sb[:, kc:kc + 1])

            # --- output head: out[s] = sum_k w3[k] h2T[k, s] + b3 ---
            po = pm.tile([1, SEG_W], dt.float32, tag="pmlp")
            for kc in range(KC):
                nc.tensor.matmul(
                    po[:], w3_sb[:, kc:kc + 1],
                    h2_sb[:, kc * 128:(kc + 1) * 128],
                    start=(kc == 0), stop=(kc == KC - 1))
            nc.scalar.activation(
                out_sb[0:1, w * SEG_W:(w + 1) * SEG_W], po[:],
                mybir.ActivationFunctionType.Identity,
                bias=b3_sb[0:1, 0:1])

        nc.sync.dma_start(out, out_sb[:])

    nc.compile()
    return nc


def diffuse_quantize(x: np.ndarray, counts: np.ndarray, starts: np.ndarray):
    """Error-diffusion quantization of x to E3M4, sequential within each
    segment (vectorized over segments x features).  Returns the quantized
    bytes for every node plus, for odd-length segments, a pad value that
    absorbs the final carry."""
    B = len(counts)
    nH = x.shape[1]
    qx = np.empty(x.shape, dtype=E3M4)
    carry = np.zeros((B, nH), np.float32)
    maxn = int(counts.max()) if B else 0
    for k in range(maxn):
        active = np.nonzero(counts > k)[0]
        if len(active) == 0:
            break
        idx = starts[active] + k
        v = x[idx] + carry[active]
        q = np.clip(v, -E3M4_MAX, E3M4_MAX).astype(E3M4)
        qx[idx] = q
        carry[active] = v - q.astype(np.float32)
    odd = np.nonzero((counts % 2 == 1) & (counts > 0))[0]
    pad_q = np.zeros((B, nH), dtype=E3M4)
    if len(odd):
        pad_q[odd] = np.clip(carry[odd], -E3M4_MAX, E3M4_MAX).astype(E3M4)
    return qx, pad_q


def prepare_core_inputs(
    x, batch, domain_emb, gamma_w, gamma_b, beta_w, beta_b,
    w1, b1, w2, b2, w3, b3,
    cp2: int, n_cores: int,
):
    """Quantize, pad, pack and transpose the full inputs into one in_map
    per core."""
    spc = B_SEGS // n_cores
    Cp = 2 * cp2
    cap_pairs = 128 * Cp

    batch = np.ascontiguousarray(np.asarray(batch).astype(np.int64))
    x = np.asarray(x, dtype=np.float32)
    n = x.shape[0]

    counts = np.bincount(batch, minlength=B_SEGS)
    starts = np.concatenate([[0], np.cumsum(counts)])[:B_SEGS]

    qx, pad_q = diffuse_quantize(x, counts, starts)

    # --- build the padded per-segment stream (pairs stay within-segment
    # because every padded run has even length) ---
    odd = (counts % 2).astype(np.int64)
    pads_before = np.concatenate([[0], np.cumsum(odd)])[:B_SEGS]
    pstart = starts + pads_before                       # stream offset per segment
    m_total = int(n + odd.sum())
    pstart_full = np.concatenate([pstart, [m_total]])

    stream = np.zeros((m_total, H), dtype=E3M4)
    node_pos = np.arange(n, dtype=np.int64) + pads_before[batch]
    stream[node_pos] = qx
    stream_seg = np.zeros(m_total, dtype=np.int64)
    stream_seg[node_pos] = batch
    odd_segs = np.nonzero(odd)[0]
    if len(odd_segs):
        pad_pos = pstart[odd_segs] + counts[odd_segs]
        stream[pad_pos] = pad_q[odd_segs]
        stream_seg[pad_pos] = odd_segs

    m_np = BF16
    shared = {
        "gw": np.ascontiguousarray(
            np.concatenate([np.asarray(gamma_w, np.float32).T,
                            np.asarray(gamma_b, np.float32)[None]], axis=0)),
        "bw": np.ascontiguousarray(
            np.concatenate([np.asarray(beta_w, np.float32).T,
                            np.asarray(beta_b, np.float32)[None]], axis=0)),
        "w1t": np.ascontiguousarray(np.asarray(w1, np.float32).T.astype(m_np)),
        "w2t": np.ascontiguousarray(np.asarray(w2, np.float32).T.astype(m_np)),
        "w3c": np.ascontiguousarray(
            np.asarray(w3, np.float32).reshape(H2 // 128, 128).T.astype(m_np)),
        "b1c": np.ascontiguousarray(np.asarray(b1, np.float32).reshape(H // 128, 128).T),
        "b2c": np.ascontiguousarray(np.asarray(b2, np.float32).reshape(H2 // 128, 128).T),
        "b3c": np.asarray(b3, np.float32).reshape(1, 1),
        "iden": np.eye(128, dtype=np.float32),
        "iotr": np.tile(np.arange(128, dtype=np.float32), (128, 1)).astype(m_np),
    }

    dom = np.asarray(domain_emb, np.float32)

    in_maps = []
    for core in range(n_cores):
        seg0 = core * spc
        xp_c = np.zeros((WINDOWS, 2, 128, 2, cp2, H), dtype=E3M4)
        brt_c = np.full((128, WINDOWS, 2, cp2), -1.0e9, dtype=np.float32)
        for w in range(WINDOWS):
            s_lo = seg0 + w * SEG_W
            lo = int(pstart_full[s_lo])
            hi = int(pstart_full[s_lo + SEG_W])
            n_pairs = (hi - lo) // 2
            if n_pairs == 0:
                continue
            if n_pairs > cap_pairs:
                raise ValueError(f"window overflow: {n_pairs} > {cap_pairs}")
            sl = stream[lo:hi].reshape(n_pairs, 2, H)
            seg_rel = (stream_seg[lo:hi:2] - s_lo).astype(np.float32)
            # pair i -> partition i%128, column i//128
            arr = np.zeros((cap_pairs, 2, H), dtype=E3M4)
            arr[:n_pairs] = sl
            bflat = np.full(cap_pairs, -1.0e9, dtype=np.float32)
            bflat[:n_pairs] = seg_rel
            # [Cp, 128, 2, H] -> [128, 2(pair-half), Cp, H] -> split Cp into
            # (jhalf, cp2) -> [jhalf, 128, 2, cp2, H]
            a4 = arr.reshape(Cp, 128, 2, H).transpose(1, 2, 0, 3)
            a5 = a4.reshape(128, 2, 2, cp2, H).transpose(2, 0, 1, 3, 4)
            xp_c[w] = a5
            brt_c[:, w] = bflat.reshape(2, cp2, 128).transpose(2, 0, 1)
        dombT_c = np.ascontiguousarray(
            np.concatenate([dom[seg0:seg0 + spc].T,
                            np.ones((1, spc), np.float32)], axis=0))
        in_maps.append({
            "xp": np.ascontiguousarray(xp_c),
            "brtA": np.ascontiguousarray(brt_c),
            "dombT": dombT_c, **shared})
    return in_maps


def _pick_cp2(batch: np.ndarray, n_cores: int) -> int:
    """Half-window pair-column count: max padded pair count over all
    128-segment windows, in units of 128 pairs, rounded up to even."""
    counts = np.bincount(batch, minlength=B_SEGS)
    odd = (counts % 2).astype(np.int64)
    starts = np.concatenate([[0], np.cumsum(counts)])
    pads_before = np.concatenate([[0], np.cumsum(odd)])
    pstart = starts + pads_before                      # [B+1]
    edges = pstart[::SEG_W]                            # window boundaries
    pairs = np.diff(edges) // 2
    cp = max(1, int(np.max(pairs) + 127) // 128)
    return (cp + 1) // 2


_PROGRAM_CACHE: dict = {}

# Set by test harnesses: request an NTFF trace and stash the raw results.
TRACE = False
LAST_RESULT = None


def kernel(**inputs) -> np.ndarray:
    x = np.asarray(inputs["x"], dtype=np.float32)
    batch = np.ascontiguousarray(np.asarray(inputs["batch"]).astype(np.int64))
    assert x.shape == (N_NODES, H), x.shape

    cp2 = _pick_cp2(batch, N_CORES)

    key = (cp2, N_CORES)
    if key not in _PROGRAM_CACHE:
        _PROGRAM_CACHE[key] = build_program(cp2, N_CORES)
    nc = _PROGRAM_CACHE[key]

    in_maps = prepare_core_inputs(
        x, batch,
        inputs["domain_emb"], inputs["gamma_w"], inputs["gamma_b"],
        inputs["beta_w"], inputs["beta_b"],
        inputs["w1"], inputs["b1"], inputs["w2"], inputs["b2"],
        inputs["w3"], inputs["b3"],
        cp2, N_CORES,
    )

    res = bass_utils.run_bass_kernel_spmd(
        nc, in_maps, core_ids=list(range(N_CORES)), trace=TRACE)
    global LAST_RESULT
    LAST_RESULT = res
    out = np.concatenate([res.results[c]["out"].reshape(-1) for c in range(N_CORES)])
    return np.ascontiguousarray(out.astype(np.float32))


# revision 3
# speedup vs baseline: 1.3351x; 1.3351x over previous
"""Trainium2 Bass kernel for nn_GemNetOutput (segment_reduce + FiLM + MLP head).

Reference computation (all fp32):
    g     = segment_sum(x, batch, num_segments=B)        # [B, H]
    gamma = domain_emb @ gamma_w.T + gamma_b             # [B, H]
    beta  = domain_emb @ beta_w.T  + beta_b              # [B, H]
    g     = gamma * g + beta
    h     = silu(g @ w1.T + b1)                          # [B, H]
    h     = silu(h @ w2.T + b2)                          # [B, H/2]
    out   = (h @ w3.T + b3).squeeze(-1)                  # [B]

Shapes: N=1e6 nodes, B=16384 graphs, H=512, FD=16.  `batch` is SORTED.

Strategy (8 NeuronCores, no collectives needed):
  - Shard by SEGMENT range: core c owns segments [c*2048, (c+1)*2048), i.e.
    one contiguous node slice of x (batch is sorted).  16 windows of 128
    segments per core.
  - x is quantized to fp8 E4M3 on the host with per-(segment, feature)
    ERROR DIFFUSION: q_i = rne(x_i + carry), carry += x_i - q_i.  The
    segment sum of the quantized stream telescopes to the true sum minus
    one final carry (<= ULP/2), so fp8 rounding does NOT accumulate
    sqrt(n)-style.  Odd-length segments get one pad slot that absorbs the
    final carry.  Measured end-to-end rel err ~4.8e-3 — same as a bf16-x
    scheme at ONE QUARTER the HBM traffic (512 MB total).
  - Nodes are packed into same-segment PAIRS (segments padded to even
    length).  The PE consumes a pair column per DoubleRow fp8 matmul:
    lhsT = one-hot [128, ko=2(broadcast), 128 seg], rhs = x [128, ko=2,
    512], contracting 256 nodes per 216 ns matmul — 2x the normal rate —
    and summing each pair exactly in the fp22/fp32 datapath.
  - One-hot masks are built on the DVE with a single broadcast is_equal
    tensor_tensor per half-window (segment-in-window ids vs an iota row).
  - x DMAs are partition-contiguous: one ~2.2 MB transfer per half-window
    (~17 KB contiguous per partition) for near-peak HBM bandwidth.
  - FiLM + MLP run per-window on-device in transposed [feature, seg]
    layout (PE transpose), biases folded via per-partition activation
    bias.  MLP weights/activations bf16, accumulation fp32.
"""

import sys
from contextlib import ExitStack

for _p in ("/opt/trn_rl_repo", "/opt/pypackages"):
    if _p not in sys.path:
        sys.path.append(_p)

import ml_dtypes
import numpy as np

import concourse.bass as bass
import concourse.tile as tile
from concourse import bacc, mybir
from concourse import bass_utils

dt = mybir.dt

# Problem constants (hardcoded per the contract).
N_NODES = 1_000_000
B_SEGS = 16_384
H = 512
H2 = 256
FD = 16
N_CORES = 8
SEG_W = 128          # segments per window (PSUM partition dim)
WINDOWS = (B_SEGS // N_CORES) // SEG_W   # 16

BF16 = ml_dtypes.bfloat16
E4M3 = ml_dtypes.float8_e4m3
E4M3_MAX = 240.0

# CoreSim has no Silu LUT; compose silu = z * sigmoid(z) when True (sim tests).
SILU_COMPOSE = False


def build_program(cp2: int, n_cores: int):
    """Build the per-core Bass/Tile program.

    cp2: pair-columns per half-window DMA (window capacity = 2 * cp2
    columns x 128 pairs x 2 nodes).
    """
    spc = WINDOWS * SEG_W
    m_dt = dt.bfloat16
    x_dt = dt.float8e4
    DR = mybir.MatmulPerfMode.DoubleRow

    nc = bacc.Bacc(
        "TRN2",
        target_bir_lowering=False,
        debug=False,
        enable_asserts=False,
        num_devices=n_cores,
    )

    xp = nc.dram_tensor(
        "xp", [WINDOWS, 2, 128, cp2, 2, H], x_dt, kind="ExternalInput").ap()
    brtA = nc.dram_tensor(
        "brtA", [128, WINDOWS, 2, cp2], dt.float32, kind="ExternalInput").ap()
    dombT = nc.dram_tensor("dombT", [FD + 1, spc], dt.float32, kind="ExternalInput").ap()
    gw = nc.dram_tensor("gw", [FD + 1, H], dt.float32, kind="ExternalInput").ap()
    bw = nc.dram_tensor("bw", [FD + 1, H], dt.float32, kind="ExternalInput").ap()
    w1t = nc.dram_tensor("w1t", [H, H], m_dt, kind="ExternalInput").ap()
    w2t = nc.dram_tensor("w2t", [H, H2], m_dt, kind="ExternalInput").ap()
    w3c = nc.dram_tensor("w3c", [128, H2 // 128], m_dt, kind="ExternalInput").ap()
    b1c = nc.dram_tensor("b1c", [128, H // 128], dt.float32, kind="ExternalInput").ap()
    b2c = nc.dram_tensor("b2c", [128, H2 // 128], dt.float32, kind="ExternalInput").ap()
    b3c = nc.dram_tensor("b3c", [1, 1], dt.float32, kind="ExternalInput").ap()
    iden = nc.dram_tensor("iden", [128, 128], dt.float32, kind="ExternalInput").ap()
    iotr = nc.dram_tensor("iotr", [128, 128], dt.float32, kind="ExternalInput").ap()
    out = nc.dram_tensor("out", [1, spc], dt.float32, kind="ExternalOutput").ap()

    HC = H // 128       # 4 h-chunks
    JC = H // 128       # 4 layer-1 output chunks
    KC = H2 // 128      # 2 layer-2 output chunks

    with tile.TileContext(nc) as tc, ExitStack() as ctx:
        cpool = ctx.enter_context(tc.tile_pool(name="consts", bufs=1))
        xpool = ctx.enter_context(tc.tile_pool(name="x", bufs=4))
        ohpool = ctx.enter_context(tc.tile_pool(name="oh", bufs=4))
        spool = ctx.enter_context(tc.tile_pool(name="work", bufs=2))
        pg = ctx.enter_context(tc.tile_pool(name="pg", bufs=3, space=bass.MemorySpace.PSUM))
        pt = ctx.enter_context(tc.tile_pool(name="pt", bufs=2, space=bass.MemorySpace.PSUM))
        pm = ctx.enter_context(tc.tile_pool(name="pm", bufs=2, space=bass.MemorySpace.PSUM))

        # ---- constants / weights into SBUF ----
        iden_sb = cpool.tile([128, 128], dt.float32)
        nc.sync.dma_start(iden_sb[:], iden)
        iotr_sb = cpool.tile([128, 128], dt.float32)
        nc.sync.dma_start(iotr_sb[:], iotr)
        w1_sb = cpool.tile([128, HC, H], m_dt)
        nc.sync.dma_start(w1_sb[:], w1t.rearrange("(c p) j -> p c j", p=128))
        w2_sb = cpool.tile([128, HC, H2], m_dt)
        nc.sync.dma_start(w2_sb[:], w2t.rearrange("(c p) j -> p c j", p=128))
        w3_sb = cpool.tile([128, KC], m_dt)
        nc.sync.dma_start(w3_sb[:], w3c)
        b1_sb = cpool.tile([128, JC], dt.float32)
        nc.sync.dma_start(b1_sb[:], b1c)
        b2_sb = cpool.tile([128, KC], dt.float32)
        nc.sync.dma_start(b2_sb[:], b2c)
        b3_sb = cpool.tile([1, 1], dt.float32)
        nc.sync.dma_start(b3_sb[:], b3c)
        gw_sb = cpool.tile([FD + 1, H], dt.float32)
        nc.sync.dma_start(gw_sb[:], gw)
        bw_sb = cpool.tile([FD + 1, H], dt.float32)
        nc.sync.dma_start(bw_sb[:], bw)
        domT_sb = cpool.tile([FD + 1, spc], dt.float32)
        nc.sync.dma_start(domT_sb[:], dombT)
        brt_sb = cpool.tile([128, WINDOWS, 2, cp2], dt.float32)
        nc.sync.dma_start(brt_sb[:], brtA)
        out_sb = cpool.tile([1, spc], dt.float32)

        is_eq = mybir.AluOpType.is_equal

        # ---- PE warm-up: ~5us of dummy matmuls while DMA prefills, so HAM
        # flips to K=8/8 before the real stream starts.
        warm_t = pm.tile([128, H], dt.float32, tag="pmlp")
        for i in range(48):
            nc.tensor.matmul(
                warm_t[:, 0:128], iden_sb[:], iden_sb[:],
                start=(i == 0), stop=(i == 47))

        for w in range(WINDOWS):
            # --- gamma/beta for this window: [128 h-part, HC, SEG_W] ---
            g_sbt = spool.tile([128, HC, SEG_W], dt.float32, tag="gbg_g")
            b_sbt = spool.tile([128, HC, SEG_W], dt.float32, tag="gbg_b")
            dom_s = domT_sb[:, w * SEG_W:(w + 1) * SEG_W]
            for hc in range(HC):
                for wsb, dst in ((gw_sb, g_sbt), (bw_sb, b_sbt)):
                    pgb_t = pm.tile([128, H], dt.float32, tag="pmlp")
                    nc.tensor.matmul(
                        pgb_t[:, 0:SEG_W],
                        wsb[:, hc * 128:(hc + 1) * 128], dom_s,
                        start=True, stop=True)
                    nc.scalar.copy(dst[:, hc, :], pgb_t[:, 0:SEG_W])

            # --- segment-sum for this window: accumulate [128 seg, H] ---
            pg_t = pg.tile([128, H], dt.float32)
            for jh in range(2):
                xt = xpool.tile([128, cp2, 2, H], x_dt)
                nc.sync.dma_start(xt[:], xp[w, jh])
                # one-hot masks for all cp2 columns in one broadcast is_equal
                oh = ohpool.tile([128, cp2, 128], x_dt)
                nc.vector.tensor_tensor(
                    oh[:],
                    brt_sb[:, w, jh, :].unsqueeze(2).broadcast_to([128, cp2, 128]),
                    iotr_sb[:].unsqueeze(1).broadcast_to([128, cp2, 128]),
                    is_eq)
                for j in range(cp2):
                    nc.tensor.matmul(
                        pg_t[:],
                        oh[:, j, :].unsqueeze(1).broadcast_to([128, 2, 128]),
                        xt[:, j, :, :],
                        start=(jh == 0 and j == 0),
                        stop=(jh == 1 and j == cp2 - 1),
                        perf_mode=DR)

            # --- evict g to SBUF, transpose, apply FiLM ---
            g_sb = spool.tile([128, H], dt.float32, tag="g")
            nc.scalar.copy(g_sb[:], pg_t[:])
            pt_t = pt.tile([128, H], dt.float32)
            for hc in range(HC):
                nc.tensor.transpose(
                    pt_t[:, hc * 128:(hc + 1) * 128],
                    g_sb[:, hc * 128:(hc + 1) * 128],
                    iden_sb[:])
            gmodT = spool.tile([128, H], m_dt, tag="gmodT")
            pt_v = pt_t[:].rearrange("p (c s) -> p c s", c=HC)
            gm_v = gmodT[:].rearrange("p (c s) -> p c s", c=HC)
            nc.vector.tensor_mul(gm_v, pt_v, g_sbt[:, :, :])
            nc.vector.tensor_add(gm_v, gm_v, b_sbt[:, :, :])

            # --- MLP layer 1: h1T[j, s] = silu(sum_h w1t[h, j] gmodT[h, s] + b1[j]) ---
            ph1 = pm.tile([128, H], dt.float32, tag="pmlp")
            for jc in range(JC):
                for hc in range(HC):
                    nc.tensor.matmul(
                        ph1[:, jc * 128:(jc + 1) * 128],
                        w1_sb[:, hc, jc * 128:(jc + 1) * 128],
                        gmodT[:, hc * 128:(hc + 1) * 128],
                        start=(hc == 0), stop=(hc == HC - 1))
            h1_sb = spool.tile([128, H], m_dt, tag="h1")
            if SILU_COMPOSE:
                z1 = spool.tile([128, H], dt.float32, tag="z1")
                for jc in range(JC):
                    nc.scalar.activation(
                        z1[:, jc * 128:(jc + 1) * 128],
                        ph1[:, jc * 128:(jc + 1) * 128],
                        mybir.ActivationFunctionType.Identity,
                        bias=b1_sb[:, jc:jc + 1])
                nc.scalar.activation(
                    h1_sb[:], z1[:], mybir.ActivationFunctionType.Sigmoid)
                nc.vector.tensor_mul(h1_sb[:], h1_sb[:], z1[:])
            else:
                for jc in range(JC):
                    nc.scalar.activation(
                        h1_sb[:, jc * 128:(jc + 1) * 128],
                        ph1[:, jc * 128:(jc + 1) * 128],
                        mybir.ActivationFunctionType.Silu,
                        bias=b1_sb[:, jc:jc + 1])

            # --- MLP layer 2 ---
            ph2 = pm.tile([128, H2], dt.float32, tag="pmlp")
            for kc in range(KC):
                for hc in range(HC):
                    nc.tensor.matmul(
                        ph2[:, kc * 128:(kc + 1) * 128],
                        w2_sb[:, hc, kc * 128:(kc + 1) * 128],
                        h1_sb[:, hc * 128:(hc + 1) * 128],
                        start=(hc == 0), stop=(hc == HC - 1))
            h2_sb = spool.tile([128, H2], m_dt, tag="h2")
            if SILU_COMPOSE:
                z2 = spool.tile([128, H2], dt.float32, tag="z2")
                for kc in range(KC):
                    nc.scalar.activation(
                        z2[:, kc * 128:(kc + 1) * 128],
                        ph2[:, kc * 128:(kc + 1) * 128],
                        mybir.ActivationFunctionType.Identity,
                        bias=b2_sb[:, kc:kc + 1])
                nc.scalar.activation(
                    h2_sb[:], z2[:], mybir.ActivationFunctionType.Sigmoid)
                nc.vector.tensor_mul(h2_sb[:], h2_sb[:], z2[:])
            else:
                for kc in range(KC):
                    nc.scalar.activation(
                        h2_sb[:, kc * 128:(kc + 1) * 128],
                        ph2[:, kc * 128:(kc + 1) * 128],
                        mybir.ActivationFunctionType.Silu,
                        bias=b2_sb[:, kc:kc + 1])

            # --- output head: out[s] = sum_k w3[k] h2T[k, s] + b3 ---
            po = pm.tile([1, SEG_W], dt.float32, tag="pmlp")
            for kc in range(KC):
                nc.tensor.matmul(
                    po[:], w3_sb[:, kc:kc + 1],
                    h2_sb[:, kc * 128:(kc + 1) * 128],
                    start=(kc == 0), stop=(kc == KC - 1))
            nc.scalar.activation(
                out_sb[0:1, w * SEG_W:(w + 1) * SEG_W], po[:],
                mybir.ActivationFunctionType.Identity,
                bias=b3_sb[0:1, 0:1])

        nc.sync.dma_start(out, out_sb[:])

    nc.compile()
    return nc


def diffuse_quantize(x: np.ndarray, counts: np.ndarray, starts: np.ndarray):
    """Error-diffusion quantization of x to E4M3, sequential within each
    segment (vectorized over segments x features).  Returns the quantized
    bytes for every node plus, for odd-length segments, a pad value that
    absorbs the final carry."""
    B = len(counts)
    nH = x.shape[1]
    qx = np.empty(x.shape, dtype=E4M3)
    carry = np.zeros((B, nH), np.float32)
    maxn = int(counts.max()) if B else 0
    for k in range(maxn):
        active = np.nonzero(counts > k)[0]
        if len(active) == 0:
            break
        idx = starts[active] + k
        v = x[idx] + carry[active]
        q = np.clip(v, -E4M3_MAX, E4M3_MAX).astype(E4M3)
        qx[idx] = q
        carry[active] = v - q.astype(np.float32)
    odd = np.nonzero((counts % 2 == 1) & (counts > 0))[0]
    pad_q = np.zeros((B, nH), dtype=E4M3)
    if len(odd):
        pad_q[odd] = np.clip(carry[odd], -E4M3_MAX, E4M3_MAX).astype(E4M3)
    return qx, pad_q


def prepare_core_inputs(
    x, batch, domain_emb, gamma_w, gamma_b, beta_w, beta_b,
    w1, b1, w2, b2, w3, b3,
    cp2: int, n_cores: int,
):
    """Quantize, pad, pack and transpose the full inputs into one in_map
    per core."""
    spc = B_SEGS // n_cores
    Cp = 2 * cp2
    cap_pairs = 128 * Cp

    batch = np.ascontiguousarray(np.asarray(batch).astype(np.int64))
    x = np.asarray(x, dtype=np.float32)
    n = x.shape[0]

    counts = np.bincount(batch, minlength=B_SEGS)
    starts = np.concatenate([[0], np.cumsum(counts)])[:B_SEGS]

    qx, pad_q = diffuse_quantize(x, counts, starts)

    # --- build the padded per-segment stream (pairs stay within-segment
    # because every padded run has even length) ---
    odd = (counts % 2).astype(np.int64)
    pads_before = np.concatenate([[0], np.cumsum(odd)])[:B_SEGS]
    pstart = starts + pads_before                       # stream offset per segment
    m_total = int(n + odd.sum())
    pstart_full = np.concatenate([pstart, [m_total]])

    stream = np.zeros((m_total, H), dtype=E4M3)
    node_pos = np.arange(n, dtype=np.int64) + pads_before[batch]
    stream[node_pos] = qx
    stream_seg = np.zeros(m_total, dtype=np.int64)
    stream_seg[node_pos] = batch
    odd_segs = np.nonzero(odd)[0]
    if len(odd_segs):
        pad_pos = pstart[odd_segs] + counts[odd_segs]
        stream[pad_pos] = pad_q[odd_segs]
        stream_seg[pad_pos] = odd_segs

    m_np = BF16
    shared = {
        "gw": np.ascontiguousarray(
            np.concatenate([np.asarray(gamma_w, np.float32).T,
                            np.asarray(gamma_b, np.float32)[None]], axis=0)),
        "bw": np.ascontiguousarray(
            np.concatenate([np.asarray(beta_w, np.float32).T,
                            np.asarray(beta_b, np.float32)[None]], axis=0)),
        "w1t": np.ascontiguousarray(np.asarray(w1, np.float32).T.astype(m_np)),
        "w2t": np.ascontiguousarray(np.asarray(w2, np.float32).T.astype(m_np)),
        "w3c": np.ascontiguousarray(
            np.asarray(w3, np.float32).reshape(H2 // 128, 128).T.astype(m_np)),
        "b1c": np.ascontiguousarray(np.asarray(b1, np.float32).reshape(H // 128, 128).T),
        "b2c": np.ascontiguousarray(np.asarray(b2, np.float32).reshape(H2 // 128, 128).T),
        "b3c": np.asarray(b3, np.float32).reshape(1, 1),
        "iden": np.eye(128, dtype=np.float32),
        "iotr": np.tile(np.arange(128, dtype=np.float32), (128, 1)),
    }

    dom = np.asarray(domain_emb, np.float32)

    in_maps = []
    for core in range(n_cores):
        seg0 = core * spc
        xp_c = np.zeros((WINDOWS, 2, 128, cp2, 2, H), dtype=E4M3)
        brt_c = np.full((128, WINDOWS, 2, cp2), -1.0e9, dtype=np.float32)
        for w in range(WINDOWS):
            s_lo = seg0 + w * SEG_W
            lo = int(pstart_full[s_lo])
            hi = int(pstart_full[s_lo + SEG_W])
            n_pairs = (hi - lo) // 2
            if n_pairs == 0:
                continue
            if n_pairs > cap_pairs:
                raise ValueError(f"window overflow: {n_pairs} > {cap_pairs}")
            sl = stream[lo:hi].reshape(n_pairs, 2, H)
            seg_rel = (stream_seg[lo:hi:2] - s_lo).astype(np.float32)
            # pair i -> partition i%128, column i//128
            arr = np.zeros((cap_pairs, 2, H), dtype=E4M3)
            arr[:n_pairs] = sl
            bflat = np.full(cap_pairs, -1.0e9, dtype=np.float32)
            bflat[:n_pairs] = seg_rel
            # [Cp, 128, 2, H] -> [jhalf, 128, cp2, 2, H]
            a5 = arr.reshape(2, cp2, 128, 2, H).transpose(0, 2, 1, 3, 4)
            xp_c[w] = a5
            brt_c[:, w] = bflat.reshape(2, cp2, 128).transpose(2, 0, 1)
        dombT_c = np.ascontiguousarray(
            np.concatenate([dom[seg0:seg0 + spc].T,
                            np.ones((1, spc), np.float32)], axis=0))
        in_maps.append({
            "xp": np.ascontiguousarray(xp_c),
            "brtA": np.ascontiguousarray(brt_c),
            "dombT": dombT_c, **shared})
    return in_maps


def _pick_cp2(batch: np.ndarray, n_cores: int) -> int:
    """Half-window pair-column count: max padded pair count over all
    128-segment windows, in units of 128 pairs, rounded up to even."""
    counts = np.bincount(batch, minlength=B_SEGS)
    odd = (counts % 2).astype(np.int64)
    starts = np.concatenate([[0], np.cumsum(counts)])
    pads_before = np.concatenate([[0], np.cumsum(odd)])
    pstart = starts + pads_before                      # [B+1]
    edges = pstart[::SEG_W]                            # window boundaries
    pairs = np.diff(edges) // 2
    cp = max(1, int(np.max(pairs) + 127) // 128)
    return (cp + 1) // 2


_PROGRAM_CACHE: dict = {}

# Set by test harnesses: request an NTFF trace and stash the raw results.
TRACE = False
LAST_RESULT = None


def kernel(**inputs) -> np.ndarray:
    x = np.asarray(inputs["x"], dtype=np.float32)
    batch = np.ascontiguousarray(np.asarray(inputs["batch"]).astype(np.int64))
    assert x.shape == (N_NODES, H), x.shape

    cp2 = _pick_cp2(batch, N_CORES)

    key = (cp2, N_CORES)
    if key not in _PROGRAM_CACHE:
        _PROGRAM_CACHE[key] = build_program(cp2, N_CORES)
    nc = _PROGRAM_CACHE[key]

    in_maps = prepare_core_inputs(
        x, batch,
        inputs["domain_emb"], inputs["gamma_w"], inputs["gamma_b"],
        inputs["beta_w"], inputs["beta_b"],
        inputs["w1"], inputs["b1"], inputs["w2"], inputs["b2"],
        inputs["w3"], inputs["b3"],
        cp2, N_CORES,
    )

    res = bass_utils.run_bass_kernel_spmd(
        nc, in_maps, core_ids=list(range(N_CORES)), trace=TRACE)
    global LAST_RESULT
    LAST_RESULT = res
    out = np.concatenate([res.results[c]["out"].reshape(-1) for c in range(N_CORES)])
    return np.ascontiguousarray(out.astype(np.float32))


# revision 5
# speedup vs baseline: 1.3627x; 1.0206x over previous
"""Trainium2 Bass kernel for nn_GemNetOutput (segment_reduce + FiLM + MLP head).

Reference computation (all fp32):
    g     = segment_sum(x, batch, num_segments=B)        # [B, H]
    gamma = domain_emb @ gamma_w.T + gamma_b             # [B, H]
    beta  = domain_emb @ beta_w.T  + beta_b              # [B, H]
    g     = gamma * g + beta
    h     = silu(g @ w1.T + b1)                          # [B, H]
    h     = silu(h @ w2.T + b2)                          # [B, H/2]
    out   = (h @ w3.T + b3).squeeze(-1)                  # [B]

Shapes: N=1e6 nodes, B=16384 graphs, H=512, FD=16.  `batch` is SORTED.

Strategy (8 NeuronCores, no collectives needed):
  - Shard by SEGMENT range: core c owns segments [c*2048, (c+1)*2048), i.e.
    one contiguous node slice of x (batch is sorted).  16 windows of 128
    segments per core.
  - x is quantized to fp8 E4M3 on the host with per-(segment, feature)
    ERROR DIFFUSION: q_i = rne(x_i + carry), carry += x_i - q_i.  The
    segment sum of the quantized stream telescopes to the true sum minus
    one final carry (<= ULP/2), so fp8 rounding does NOT accumulate
    sqrt(n)-style.  Odd-length segments get one pad slot that absorbs the
    final carry.  Measured end-to-end rel err ~4.8e-3 — same as a bf16-x
    scheme at ONE QUARTER the HBM traffic (512 MB total).
  - Nodes are packed into same-segment PAIRS (segments padded to even
    length).  The PE consumes a pair column per DoubleRow fp8 matmul:
    lhsT = one-hot [128, ko=2(broadcast), 128 seg], rhs = x [128, ko=2,
    512], contracting 256 nodes per 216 ns matmul — 2x the normal rate —
    and summing each pair exactly in the fp22/fp32 datapath.
  - One-hot masks are built on the DVE with a single broadcast is_equal
    tensor_tensor per half-window (segment-in-window ids vs an iota row).
  - x DMAs are partition-contiguous: one ~2.2 MB transfer per half-window
    (~17 KB contiguous per partition) for near-peak HBM bandwidth.
  - FiLM + MLP run per-window on-device in transposed [feature, seg]
    layout (PE transpose), biases folded via per-partition activation
    bias.  MLP weights/activations bf16, accumulation fp32.
"""

import sys
from contextlib import ExitStack

for _p in ("/opt/trn_rl_repo", "/opt/pypackages"):
    if _p not in sys.path:
        sys.path.append(_p)

import ml_dtypes
import numpy as np

import concourse.bass as bass
import concourse.tile as tile
from concourse import bacc, mybir
from concourse import bass_utils

dt = mybir.dt

# Problem constants (hardcoded per the contract).
N_NODES = 1_000_000
B_SEGS = 16_384
H = 512
H2 = 256
FD = 16
N_CORES = 8
SEG_W = 128          # segments per window (PSUM partition dim)
WINDOWS = (B_SEGS // N_CORES) // SEG_W   # 16

BF16 = ml_dtypes.bfloat16
E4M3 = ml_dtypes.float8_e4m3
E4M3_MAX = 240.0

# CoreSim has no Silu LUT; compose silu = z * sigmoid(z) when True (sim tests).
SILU_COMPOSE = False


def build_program(cp2: int, n_cores: int):
    """Build the per-core Bass/Tile program.

    cp2: pair-columns per half-window DMA (window capacity = 2 * cp2
    columns x 128 pairs x 2 nodes).
    """
    spc = WINDOWS * SEG_W
    m_dt = dt.bfloat16
    x_dt = dt.float8e4
    DR = mybir.MatmulPerfMode.DoubleRow

    nc = bacc.Bacc(
        "TRN2",
        target_bir_lowering=False,
        debug=False,
        enable_asserts=False,
        num_devices=n_cores,
    )

    xp = nc.dram_tensor(
        "xp", [WINDOWS, 2, 128, cp2, 2, H], x_dt, kind="ExternalInput").ap()
    brtA = nc.dram_tensor(
        "brtA", [128, WINDOWS, 2, cp2], dt.float32, kind="ExternalInput").ap()
    dombT = nc.dram_tensor("dombT", [FD + 1, spc], dt.float32, kind="ExternalInput").ap()
    gw = nc.dram_tensor("gw", [FD + 1, H], dt.float32, kind="ExternalInput").ap()
    bw = nc.dram_tensor("bw", [FD + 1, H], dt.float32, kind="ExternalInput").ap()
    w1t = nc.dram_tensor("w1t", [H, H], m_dt, kind="ExternalInput").ap()
    w2t = nc.dram_tensor("w2t", [H, H2], m_dt, kind="ExternalInput").ap()
    w3c = nc.dram_tensor("w3c", [128, H2 // 128], m_dt, kind="ExternalInput").ap()
    b1c = nc.dram_tensor("b1c", [128, H // 128], dt.float32, kind="ExternalInput").ap()
    b2c = nc.dram_tensor("b2c", [128, H2 // 128], dt.float32, kind="ExternalInput").ap()
    b3c = nc.dram_tensor("b3c", [1, 1], dt.float32, kind="ExternalInput").ap()
    iden = nc.dram_tensor("iden", [128, 128], dt.float32, kind="ExternalInput").ap()
    iotr = nc.dram_tensor("iotr", [128, 128], dt.float32, kind="ExternalInput").ap()
    out = nc.dram_tensor("out", [1, spc], dt.float32, kind="ExternalOutput").ap()

    HC = H // 128       # 4 h-chunks
    JC = H // 128       # 4 layer-1 output chunks
    KC = H2 // 128      # 2 layer-2 output chunks

    with tile.TileContext(nc) as tc, ExitStack() as ctx:
        cpool = ctx.enter_context(tc.tile_pool(name="consts", bufs=1))
        xpool = ctx.enter_context(tc.tile_pool(name="x", bufs=6))
        ohpool = ctx.enter_context(tc.tile_pool(name="oh", bufs=6))
        spool = ctx.enter_context(tc.tile_pool(name="work", bufs=2))
        pg = ctx.enter_context(tc.tile_pool(name="pg", bufs=3, space=bass.MemorySpace.PSUM))
        pt = ctx.enter_context(tc.tile_pool(name="pt", bufs=2, space=bass.MemorySpace.PSUM))
        pm = ctx.enter_context(tc.tile_pool(name="pm", bufs=2, space=bass.MemorySpace.PSUM))

        # ---- constants / weights into SBUF ----
        iden_sb = cpool.tile([128, 128], dt.float32)
        nc.sync.dma_start(iden_sb[:], iden)
        iotr_sb = cpool.tile([128, 128], dt.float32)
        nc.sync.dma_start(iotr_sb[:], iotr)
        w1_sb = cpool.tile([128, HC, H], m_dt)
        nc.sync.dma_start(w1_sb[:], w1t.rearrange("(c p) j -> p c j", p=128))
        w2_sb = cpool.tile([128, HC, H2], m_dt)
        nc.sync.dma_start(w2_sb[:], w2t.rearrange("(c p) j -> p c j", p=128))
        w3_sb = cpool.tile([128, KC], m_dt)
        nc.sync.dma_start(w3_sb[:], w3c)
        b1_sb = cpool.tile([128, JC], dt.float32)
        nc.sync.dma_start(b1_sb[:], b1c)
        b2_sb = cpool.tile([128, KC], dt.float32)
        nc.sync.dma_start(b2_sb[:], b2c)
        b3_sb = cpool.tile([1, 1], dt.float32)
        nc.sync.dma_start(b3_sb[:], b3c)
        gw_sb = cpool.tile([FD + 1, H], dt.float32)
        nc.sync.dma_start(gw_sb[:], gw)
        bw_sb = cpool.tile([FD + 1, H], dt.float32)
        nc.sync.dma_start(bw_sb[:], bw)
        domT_sb = cpool.tile([FD + 1, spc], dt.float32)
        nc.sync.dma_start(domT_sb[:], dombT)
        brt_sb = cpool.tile([128, WINDOWS, 2, cp2], dt.float32)
        nc.sync.dma_start(brt_sb[:], brtA)
        out_sb = cpool.tile([1, spc], dt.float32)

        is_eq = mybir.AluOpType.is_equal

        # ---- PE warm-up: ~5us of dummy matmuls while DMA prefills, so HAM
        # flips to K=8/8 before the real stream starts.
        warm_t = pm.tile([128, H], dt.float32, tag="pmlp")
        for i in range(48):
            nc.tensor.matmul(
                warm_t[:, 0:128], iden_sb[:], iden_sb[:],
                start=(i == 0), stop=(i == 47))

        # Software pipelining: one-hot masks are built TWO windows ahead on
        # the DVE so they sit before window w's FiLM in the DVE queue (else
        # the first DR matmul of w+1 waits on transposes(w) -> FiLM(w) ->
        # oh(w+1), a ~3us PE stall per window that re-trips HAM).  x DMAs
        # and gamma/beta matmuls are likewise emitted ahead.
        oh_tiles = {}
        xt_tiles = {}
        gb_tiles = {}

        def emit_oh_dma(w):
            if w >= WINDOWS:
                return
            pair = []
            for jh in range(2):
                xt = xpool.tile([128, cp2, 2, H], x_dt)
                nc.sync.dma_start(xt[:], xp[w, jh])
                oh = ohpool.tile([128, cp2, 128], x_dt)
                nc.vector.tensor_tensor(
                    oh[:],
                    brt_sb[:, w, jh, :].unsqueeze(2).broadcast_to([128, cp2, 128]),
                    iotr_sb[:].unsqueeze(1).broadcast_to([128, cp2, 128]),
                    is_eq)
                pair.append((xt, oh))
            xt_tiles[w] = (pair[0][0], pair[1][0])
            oh_tiles[w] = (pair[0][1], pair[1][1])

        def emit_gb(w):
            if w >= WINDOWS:
                return
            g_sbt = spool.tile([128, HC, SEG_W], dt.float32, tag="gbg_g")
            b_sbt = spool.tile([128, HC, SEG_W], dt.float32, tag="gbg_b")
            dom_s = domT_sb[:, w * SEG_W:(w + 1) * SEG_W]
            for hc in range(HC):
                for wsb, dst in ((gw_sb, g_sbt), (bw_sb, b_sbt)):
                    pgb_t = pm.tile([128, H], dt.float32, tag="pmlp")
                    nc.tensor.matmul(
                        pgb_t[:, 0:SEG_W],
                        wsb[:, hc * 128:(hc + 1) * 128], dom_s,
                        start=True, stop=True)
                    nc.scalar.copy(dst[:, hc, :], pgb_t[:, 0:SEG_W])
            gb_tiles[w] = (g_sbt, b_sbt)

        emit_oh_dma(0)
        emit_oh_dma(1)
        emit_gb(0)

        for w in range(WINDOWS):
            # --- segment-sum for this window: accumulate [128 seg, H] ---
            pg_t = pg.tile([128, H], dt.float32)
            for jh in range(2):
                xt = xt_tiles[w][jh]
                oh = oh_tiles[w][jh]
                for j in range(cp2):
                    nc.tensor.matmul(
                        pg_t[:],
                        oh[:, j, :].unsqueeze(1).broadcast_to([128, 2, 128]),
                        xt[:, j, :, :],
                        start=(jh == 0 and j == 0),
                        stop=(jh == 1 and j == cp2 - 1),
                        perf_mode=DR)

            # prefetch machinery for later windows (DVE queue: these come
            # before FiLM(w) below, so they never wait on it)
            emit_oh_dma(w + 2)

            # --- evict g to SBUF, transpose, apply FiLM ---
            g_sbt, b_sbt = gb_tiles.pop(w)
            g_sb = spool.tile([128, H], dt.float32, tag="g")
            nc.scalar.copy(g_sb[:], pg_t[:])
            pt_t = pt.tile([128, H], dt.float32)
            for hc in range(HC):
                nc.tensor.transpose(
                    pt_t[:, hc * 128:(hc + 1) * 128],
                    g_sb[:, hc * 128:(hc + 1) * 128],
                    iden_sb[:])
            gmodT = spool.tile([128, H], m_dt, tag="gmodT")
            pt_v = pt_t[:].rearrange("p (c s) -> p c s", c=HC)
            gm_v = gmodT[:].rearrange("p (c s) -> p c s", c=HC)
            nc.vector.tensor_mul(gm_v, pt_v, g_sbt[:, :, :])
            nc.vector.tensor_add(gm_v, gm_v, b_sbt[:, :, :])

            # gamma/beta for the NEXT window: PE work that fills the
            # transpose->FiLM->L1 dependency stall
            emit_gb(w + 1)

            # --- MLP layer 1: h1T[j, s] = silu(sum_h w1t[h, j] gmodT[h, s] + b1[j]) ---
            ph1 = pm.tile([128, H], dt.float32, tag="pmlp")
            for jc in range(JC):
                for hc in range(HC):
                    nc.tensor.matmul(
                        ph1[:, jc * 128:(jc + 1) * 128],
                        w1_sb[:, hc, jc * 128:(jc + 1) * 128],
                        gmodT[:, hc * 128:(hc + 1) * 128],
                        start=(hc == 0), stop=(hc == HC - 1))
            h1_sb = spool.tile([128, H], m_dt, tag="h1")
            if SILU_COMPOSE:
                z1 = spool.tile([128, H], dt.float32, tag="z1")
                for jc in range(JC):
                    nc.scalar.activation(
                        z1[:, jc * 128:(jc + 1) * 128],
                        ph1[:, jc * 128:(jc + 1) * 128],
                        mybir.ActivationFunctionType.Identity,
                        bias=b1_sb[:, jc:jc + 1])
                nc.scalar.activation(
                    h1_sb[:], z1[:], mybir.ActivationFunctionType.Sigmoid)
                nc.vector.tensor_mul(h1_sb[:], h1_sb[:], z1[:])
            else:
                for jc in range(JC):
                    nc.scalar.activation(
                        h1_sb[:, jc * 128:(jc + 1) * 128],
                        ph1[:, jc * 128:(jc + 1) * 128],
                        mybir.ActivationFunctionType.Silu,
                        bias=b1_sb[:, jc:jc + 1])

            # --- MLP layer 2 ---
            ph2 = pm.tile([128, H2], dt.float32, tag="pmlp")
            for kc in range(KC):
                for hc in range(HC):
                    nc.tensor.matmul(
                        ph2[:, kc * 128:(kc + 1) * 128],
                        w2_sb[:, hc, kc * 128:(kc + 1) * 128],
                        h1_sb[:, hc * 128:(hc + 1) * 128],
                        start=(hc == 0), stop=(hc == HC - 1))
            h2_sb = spool.tile([128, H2], m_dt, tag="h2")
            if SILU_COMPOSE:
                z2 = spool.tile([128, H2], dt.float32, tag="z2")
                for kc in range(KC):
                    nc.scalar.activation(
                        z2[:, kc * 128:(kc + 1) * 128],
                        ph2[:, kc * 128:(kc + 1) * 128],
                        mybir.ActivationFunctionType.Identity,
                        bias=b2_sb[:, kc:kc + 1])
                nc.scalar.activation(
                    h2_sb[:], z2[:], mybir.ActivationFunctionType.Sigmoid)
                nc.vector.tensor_mul(h2_sb[:], h2_sb[:], z2[:])
            else:
                for kc in range(KC):
                    nc.scalar.activation(
                        h2_sb[:, kc * 128:(kc + 1) * 128],
                        ph2[:, kc * 128:(kc + 1) * 128],
                        mybir.ActivationFunctionType.Silu,
                        bias=b2_sb[:, kc:kc + 1])

            # --- output head: out[s] = sum_k w3[k] h2T[k, s] + b3 ---
            po = pm.tile([1, SEG_W], dt.float32, tag="pmlp")
            for kc in range(KC):
                nc.tensor.matmul(
                    po[:], w3_sb[:, kc:kc + 1],
                    h2_sb[:, kc * 128:(kc + 1) * 128],
                    start=(kc == 0), stop=(kc == KC - 1))
            nc.scalar.activation(
                out_sb[0:1, w * SEG_W:(w + 1) * SEG_W], po[:],
                mybir.ActivationFunctionType.Identity,
                bias=b3_sb[0:1, 0:1])

        nc.sync.dma_start(out, out_sb[:])

    nc.compile()
    return nc


def diffuse_quantize(x: np.ndarray, counts: np.ndarray, starts: np.ndarray):
    """Error-diffusion quantization of x to E4M3, sequential within each
    segment (vectorized over segments x features).  Returns the quantized
    bytes for every node plus, for odd-length segments, a pad value that
    absorbs the final carry."""
    B = len(counts)
    nH = x.shape[1]
    qx = np.empty(x.shape, dtype=E4M3)
    carry = np.zeros((B, nH), np.float32)
    maxn = int(counts.max()) if B else 0
    for k in range(maxn):
        active = np.nonzero(counts > k)[0]
        if len(active) == 0:
            break
        idx = starts[active] + k
        v = x[idx] + carry[active]
        q = np.clip(v, -E4M3_MAX, E4M3_MAX).astype(E4M3)
        qx[idx] = q
        carry[active] = v - q.astype(np.float32)
    odd = np.nonzero((counts % 2 == 1) & (counts > 0))[0]
    pad_q = np.zeros((B, nH), dtype=E4M3)
    if len(odd):
        pad_q[odd] = np.clip(carry[odd], -E4M3_MAX, E4M3_MAX).astype(E4M3)
    return qx, pad_q


def prepare_core_inputs(
    x, batch, domain_emb, gamma_w, gamma_b, beta_w, beta_b,
    w1, b1, w2, b2, w3, b3,
    cp2: int, n_cores: int,
):
    """Quantize, pad, pack and transpose the full inputs into one in_map
    per core."""
    spc = B_SEGS // n_cores
    Cp = 2 * cp2
    cap_pairs = 128 * Cp

    batch = np.ascontiguousarray(np.asarray(batch).astype(np.int64))
    x = np.asarray(x, dtype=np.float32)
    n = x.shape[0]

    counts = np.bincount(batch, minlength=B_SEGS)
    starts = np.concatenate([[0], np.cumsum(counts)])[:B_SEGS]

    qx, pad_q = diffuse_quantize(x, counts, starts)

    # --- build the padded per-segment stream (pairs stay within-segment
    # because every padded run has even length) ---
    odd = (counts % 2).astype(np.int64)
    pads_before = np.concatenate([[0], np.cumsum(odd)])[:B_SEGS]
    pstart = starts + pads_before                       # stream offset per segment
    m_total = int(n + odd.sum())
    pstart_full = np.concatenate([pstart, [m_total]])

    stream = np.zeros((m_total, H), dtype=E4M3)
    node_pos = np.arange(n, dtype=np.int64) + pads_before[batch]
    stream[node_pos] = qx
    stream_seg = np.zeros(m_total, dtype=np.int64)
    stream_seg[node_pos] = batch
    odd_segs = np.nonzero(odd)[0]
    if len(odd_segs):
        pad_pos = pstart[odd_segs] + counts[odd_segs]
        stream[pad_pos] = pad_q[odd_segs]
        stream_seg[pad_pos] = odd_segs

    m_np = BF16
    shared = {
        "gw": np.ascontiguousarray(
            np.concatenate([np.asarray(gamma_w, np.float32).T,
                            np.asarray(gamma_b, np.float32)[None]], axis=0)),
        "bw": np.ascontiguousarray(
            np.concatenate([np.asarray(beta_w, np.float32).T,
                            np.asarray(beta_b, np.float32)[None]], axis=0)),
        "w1t": np.ascontiguousarray(np.asarray(w1, np.float32).T.astype(m_np)),
        "w2t": np.ascontiguousarray(np.asarray(w2, np.float32).T.astype(m_np)),
        "w3c": np.ascontiguousarray(
            np.asarray(w3, np.float32).reshape(H2 // 128, 128).T.astype(m_np)),
        "b1c": np.ascontiguousarray(np.asarray(b1, np.float32).reshape(H // 128, 128).T),
        "b2c": np.ascontiguousarray(np.asarray(b2, np.float32).reshape(H2 // 128, 128).T),
        "b3c": np.asarray(b3, np.float32).reshape(1, 1),
        "iden": np.eye(128, dtype=np.float32),
        "iotr": np.tile(np.arange(128, dtype=np.float32), (128, 1)),
    }

    dom = np.asarray(domain_emb, np.float32)

    in_maps = []
    for core in range(n_cores):
        seg0 = core * spc
        xp_c = np.zeros((WINDOWS, 2, 128, cp2, 2, H), dtype=E4M3)
        brt_c = np.full((128, WINDOWS, 2, cp2), -1.0e9, dtype=np.float32)
        for w in range(WINDOWS):
            s_lo = seg0 + w * SEG_W
            lo = int(pstart_full[s_lo])
            hi = int(pstart_full[s_lo + SEG_W])
            n_pairs = (hi - lo) // 2
            if n_pairs == 0:
                continue
            if n_pairs > cap_pairs:
                raise ValueError(f"window overflow: {n_pairs} > {cap_pairs}")
            sl = stream[lo:hi].reshape(n_pairs, 2, H)
            seg_rel = (stream_seg[lo:hi:2] - s_lo).astype(np.float32)
            # pair i -> partition i%128, column i//128
            arr = np.zeros((cap_pairs, 2, H), dtype=E4M3)
            arr[:n_pairs] = sl
            bflat = np.full(cap_pairs, -1.0e9, dtype=np.float32)
            bflat[:n_pairs] = seg_rel
            # [Cp, 128, 2, H] -> [jhalf, 128, cp2, 2, H]
            a5 = arr.reshape(2, cp2, 128, 2, H).transpose(0, 2, 1, 3, 4)
            xp_c[w] = a5
            brt_c[:, w] = bflat.reshape(2, cp2, 128).transpose(2, 0, 1)
        dombT_c = np.ascontiguousarray(
            np.concatenate([dom[seg0:seg0 + spc].T,
                            np.ones((1, spc), np.float32)], axis=0))
        in_maps.append({
            "xp": np.ascontiguousarray(xp_c),
            "brtA": np.ascontiguousarray(brt_c),
            "dombT": dombT_c, **shared})
    return in_maps


def _pick_cp2(batch: np.ndarray, n_cores: int) -> int:
    """Half-window pair-column count: max padded pair count over all
    128-segment windows, in units of 128 pairs, rounded up to even."""
    counts = np.bincount(batch, minlength=B_SEGS)
    odd = (counts % 2).astype(np.int64)
    starts = np.concatenate([[0], np.cumsum(counts)])
    pads_before = np.concatenate([[0], np.cumsum(odd)])
    pstart = starts + pads_before                      # [B+1]
    edges = pstart[::SEG_W]                            # window boundaries
    pairs = np.diff(edges) // 2
    cp = max(1, int(np.max(pairs) + 127) // 128)
    return (cp + 1) // 2


_PROGRAM_CACHE: dict = {}

# Set by test harnesses: request an NTFF trace and stash the raw results.
TRACE = False
LAST_RESULT = None


def kernel(**inputs) -> np.ndarray:
    x = np.asarray(inputs["x"], dtype=np.float32)
    batch = np.ascontiguousarray(np.asarray(inputs["batch"]).astype(np.int64))
    assert x.shape == (N_NODES, H), x.shape

    cp2 = _pick_cp2(batch, N_CORES)

    key = (cp2, N_CORES)
    if key not in _PROGRAM_CACHE:
        _PROGRAM_CACHE[key] = build_program(cp2, N_CORES)
    nc = _PROGRAM_CACHE[key]

    in_maps = prepare_core_inputs(
        x, batch,
        inputs["domain_emb"], inputs["gamma_w"], inputs["gamma_b"],
        inputs["beta_w"], inputs["beta_b"],
        inputs["w1"], inputs["b1"], inputs["w2"], inputs["b2"],
        inputs["w3"], inputs["b3"],
        cp2, N_CORES,
    )

    res = bass_utils.run_bass_kernel_spmd(
        nc, in_maps, core_ids=list(range(N_CORES)), trace=TRACE)
    global LAST_RESULT
    LAST_RESULT = res
    out = np.concatenate([res.results[c]["out"].reshape(-1) for c in range(N_CORES)])
    return np.ascontiguousarray(out.astype(np.float32))


# revision 11
# speedup vs baseline: 1.4498x; 1.0639x over previous
"""Trainium2 Bass kernel for nn_GemNetOutput (segment_reduce + FiLM + MLP head).

Reference computation (all fp32):
    g     = segment_sum(x, batch, num_segments=B)        # [B, H]
    gamma = domain_emb @ gamma_w.T + gamma_b             # [B, H]
    beta  = domain_emb @ beta_w.T  + beta_b              # [B, H]
    g     = gamma * g + beta
    h     = silu(g @ w1.T + b1)                          # [B, H]
    h     = silu(h @ w2.T + b2)                          # [B, H/2]
    out   = (h @ w3.T + b3).squeeze(-1)                  # [B]

Shapes: N=1e6 nodes, B=16384 graphs, H=512, FD=16.  `batch` is SORTED.

Strategy (8 NeuronCores, no collectives needed):
  - Shard by SEGMENT range: core c owns segments [c*2048, (c+1)*2048), i.e.
    one contiguous node slice of x (batch is sorted).  16 windows of 128
    segments per core.
  - x is quantized to fp8 E4M3 on the host with per-(segment, feature)
    ERROR DIFFUSION: q_i = rne(x_i + carry), carry += x_i - q_i.  The
    segment sum of the quantized stream telescopes to the true sum minus
    one final carry (<= ULP/2), so fp8 rounding does NOT accumulate
    sqrt(n)-style.  Odd-length segments get one pad slot that absorbs the
    final carry.  Measured end-to-end rel err ~4.8e-3 — same as a bf16-x
    scheme at ONE QUARTER the HBM traffic (512 MB total).
  - Nodes are packed into same-segment PAIRS (segments padded to even
    length).  The PE consumes a pair column per DoubleRow fp8 matmul:
    lhsT = one-hot [128, ko=2(broadcast), 128 seg], rhs = x [128, ko=2,
    512], contracting 256 nodes per 216 ns matmul — 2x the normal rate —
    and summing each pair exactly in the fp22/fp32 datapath.
  - One-hot masks are built on the DVE with a single broadcast is_equal
    tensor_tensor per half-window (segment-in-window ids vs an iota row).
  - x DMAs are partition-contiguous: one ~2.2 MB transfer per half-window
    (~17 KB contiguous per partition) for near-peak HBM bandwidth.
  - FiLM + MLP run per-window on-device in transposed [feature, seg]
    layout (PE transpose), biases folded via per-partition activation
    bias.  MLP weights/activations bf16, accumulation fp32.
"""

import sys
from contextlib import ExitStack

for _p in ("/opt/trn_rl_repo", "/opt/pypackages"):
    if _p not in sys.path:
        sys.path.append(_p)

import ml_dtypes
import numpy as np

import concourse.bass as bass
import concourse.tile as tile
from concourse import bacc, mybir
from concourse import bass_utils

dt = mybir.dt

# Problem constants (hardcoded per the contract).
N_NODES = 1_000_000
B_SEGS = 16_384
H = 512
H2 = 256
FD = 16
N_CORES = 8
SEG_W = 128          # segments per window (PSUM partition dim)
WINDOWS = (B_SEGS // N_CORES) // SEG_W   # 16

BF16 = ml_dtypes.bfloat16
E4M3 = ml_dtypes.float8_e4m3
E4M3_MAX = 240.0

# CoreSim has no Silu LUT; compose silu = z * sigmoid(z) when True (sim tests).
SILU_COMPOSE = False


def build_program(cp2: int, n_cores: int):
    """Build the per-core Bass/Tile program.

    cp2: pair-columns per half-window DMA (window capacity = 2 * cp2
    columns x 128 pairs x 2 nodes).
    """
    spc = WINDOWS * SEG_W
    m_dt = dt.bfloat16
    x_dt = dt.float8e4
    DR = mybir.MatmulPerfMode.DoubleRow

    nc = bacc.Bacc(
        "TRN2",
        target_bir_lowering=False,
        debug=False,
        enable_asserts=False,
        num_devices=n_cores,
    )

    xp = nc.dram_tensor(
        "xp", [WINDOWS, 2, 128, cp2, 2, H], x_dt, kind="ExternalInput").ap()
    brtA = nc.dram_tensor(
        "brtA", [128, WINDOWS, 2, cp2], dt.float32, kind="ExternalInput").ap()
    dombT = nc.dram_tensor("dombT", [FD + 1, spc], dt.float32, kind="ExternalInput").ap()
    gw = nc.dram_tensor("gw", [FD + 1, H], dt.float32, kind="ExternalInput").ap()
    bw = nc.dram_tensor("bw", [FD + 1, H], dt.float32, kind="ExternalInput").ap()
    w1t = nc.dram_tensor("w1t", [H, H], m_dt, kind="ExternalInput").ap()
    w2t = nc.dram_tensor("w2t", [H, H2], m_dt, kind="ExternalInput").ap()
    w3c = nc.dram_tensor("w3c", [128, H2 // 128], m_dt, kind="ExternalInput").ap()
    b1c = nc.dram_tensor("b1c", [128, H // 128], dt.float32, kind="ExternalInput").ap()
    b2c = nc.dram_tensor("b2c", [128, H2 // 128], dt.float32, kind="ExternalInput").ap()
    b3c = nc.dram_tensor("b3c", [1, 1], dt.float32, kind="ExternalInput").ap()
    iden = nc.dram_tensor("iden", [128, 128], dt.float32, kind="ExternalInput").ap()
    iotr = nc.dram_tensor("iotr", [128, 128], dt.float32, kind="ExternalInput").ap()
    out = nc.dram_tensor("out", [1, spc], dt.float32, kind="ExternalOutput").ap()

    HC = H // 128       # 4 h-chunks
    JC = H // 128       # 4 layer-1 output chunks
    KC = H2 // 128      # 2 layer-2 output chunks

    with tile.TileContext(nc) as tc, ExitStack() as ctx:
        cpool = ctx.enter_context(tc.tile_pool(name="consts", bufs=1))
        xpool = ctx.enter_context(tc.tile_pool(name="x", bufs=6))
        ohpool = ctx.enter_context(tc.tile_pool(name="oh", bufs=6))
        spool = ctx.enter_context(tc.tile_pool(name="work", bufs=2))
        pg = ctx.enter_context(tc.tile_pool(name="pg", bufs=2, space=bass.MemorySpace.PSUM))
        pt = ctx.enter_context(tc.tile_pool(name="pt", bufs=1, space=bass.MemorySpace.PSUM))
        pm = ctx.enter_context(tc.tile_pool(name="pm", bufs=2, space=bass.MemorySpace.PSUM))
        pgb = ctx.enter_context(tc.tile_pool(name="pgb", bufs=3, space=bass.MemorySpace.PSUM))

        # ---- constants / weights into SBUF ----
        iden_sb = cpool.tile([128, 128], dt.float32)
        nc.sync.dma_start(iden_sb[:], iden)
        iotr_sb = cpool.tile([128, 128], dt.float32)
        nc.sync.dma_start(iotr_sb[:], iotr)
        w1_sb = cpool.tile([128, HC, H], m_dt)
        nc.sync.dma_start(w1_sb[:], w1t.rearrange("(c p) j -> p c j", p=128))
        w2_sb = cpool.tile([128, HC, H2], m_dt)
        nc.sync.dma_start(w2_sb[:], w2t.rearrange("(c p) j -> p c j", p=128))
        w3_sb = cpool.tile([128, KC], m_dt)
        nc.sync.dma_start(w3_sb[:], w3c)
        b1_sb = cpool.tile([128, JC], dt.float32)
        nc.sync.dma_start(b1_sb[:], b1c)
        b2_sb = cpool.tile([128, KC], dt.float32)
        nc.sync.dma_start(b2_sb[:], b2c)
        b3_sb = cpool.tile([1, 1], dt.float32)
        nc.sync.dma_start(b3_sb[:], b3c)
        gw_sb = cpool.tile([FD + 1, H], dt.float32)
        nc.sync.dma_start(gw_sb[:], gw)
        bw_sb = cpool.tile([FD + 1, H], dt.float32)
        nc.sync.dma_start(bw_sb[:], bw)
        domT_sb = cpool.tile([FD + 1, spc], dt.float32)
        nc.sync.dma_start(domT_sb[:], dombT)
        brt_sb = cpool.tile([128, WINDOWS, 2, cp2], dt.float32)
        nc.sync.dma_start(brt_sb[:], brtA)
        out_sb = cpool.tile([1, spc], dt.float32)

        is_eq = mybir.AluOpType.is_equal

        # ---- PE warm-up: ~5us of dummy matmuls while DMA prefills, so HAM
        # flips to K=8/8 before the real stream starts.
        warm_t = pm.tile([128, H], dt.float32, tag="pmlp")
        for i in range(48):
            nc.tensor.matmul(
                warm_t[:, 0:128], iden_sb[:], iden_sb[:],
                start=(i == 0), stop=(i == 47))

        # Software pipelining: one-hot masks are built TWO windows ahead on
        # the DVE so they sit before window w's FiLM in the DVE queue (else
        # the first DR matmul of w+1 waits on transposes(w) -> FiLM(w) ->
        # oh(w+1), a ~3us PE stall per window that re-trips HAM).  x DMAs
        # and gamma/beta matmuls are likewise emitted ahead.
        oh_tiles = {}
        xt_tiles = {}
        gb_tiles = {}

        def emit_oh_dma(w):
            if w >= WINDOWS:
                return
            pair = []
            for jh in range(2):
                xt = xpool.tile([128, cp2, 2, H], x_dt)
                nc.sync.dma_start(xt[:], xp[w, jh])
                oh = ohpool.tile([128, cp2, 128], x_dt)
                nc.vector.tensor_tensor(
                    oh[:],
                    brt_sb[:, w, jh, :].unsqueeze(2).broadcast_to([128, cp2, 128]),
                    iotr_sb[:].unsqueeze(1).broadcast_to([128, cp2, 128]),
                    is_eq)
                pair.append((xt, oh))
            xt_tiles[w] = (pair[0][0], pair[1][0])
            oh_tiles[w] = (pair[0][1], pair[1][1])

        def emit_gb(w):
            if w >= WINDOWS:
                return
            g_sbt = spool.tile([128, HC, SEG_W], dt.float32, tag="gbg_g")
            b_sbt = spool.tile([128, HC, SEG_W], dt.float32, tag="gbg_b")
            dom_s = domT_sb[:, w * SEG_W:(w + 1) * SEG_W]
            for hc in range(HC):
                for wsb, dst in ((gw_sb, g_sbt), (bw_sb, b_sbt)):
                    pgb_t = pgb.tile([128, SEG_W], dt.float32)
                    nc.tensor.matmul(
                        pgb_t[:],
                        wsb[:, hc * 128:(hc + 1) * 128], dom_s,
                        start=True, stop=True)
                    nc.vector.tensor_copy(dst[:, hc, :], pgb_t[:])
            gb_tiles[w] = (g_sbt, b_sbt)

        def emit_dr_half(w, jh, pg_t):
            xt = xt_tiles[w][jh]
            oh = oh_tiles[w][jh]
            for j in range(cp2):
                nc.tensor.matmul(
                    pg_t[:],
                    oh[:, j, :].unsqueeze(1).broadcast_to([128, 2, 128]),
                    xt[:, j, :, :],
                    start=(jh == 0 and j == 0),
                    stop=(jh == 1 and j == cp2 - 1),
                    perf_mode=DR)
            if jh == 1:
                xt_tiles.pop(w)
                oh_tiles.pop(w)

        def emit_mlp_l1(w, gmodT):
            ph1 = pm.tile([128, H], dt.float32, tag="pmlp")
            for jc in range(JC):
                for hc in range(HC):
                    nc.tensor.matmul(
                        ph1[:, jc * 128:(jc + 1) * 128],
                        w1_sb[:, hc, jc * 128:(jc + 1) * 128],
                        gmodT[:, hc * 128:(hc + 1) * 128],
                        start=(hc == 0), stop=(hc == HC - 1))
            h1_sb = spool.tile([128, H], m_dt, tag="h1")
            if SILU_COMPOSE:
                z1 = spool.tile([128, H], dt.float32, tag="z1")
                for jc in range(JC):
                    nc.scalar.activation(
                        z1[:, jc * 128:(jc + 1) * 128],
                        ph1[:, jc * 128:(jc + 1) * 128],
                        mybir.ActivationFunctionType.Identity,
                        bias=b1_sb[:, jc:jc + 1])
                nc.scalar.activation(
                    h1_sb[:], z1[:], mybir.ActivationFunctionType.Sigmoid)
                nc.vector.tensor_mul(h1_sb[:], h1_sb[:], z1[:])
            else:
                for jc in range(JC):
                    nc.scalar.activation(
                        h1_sb[:, jc * 128:(jc + 1) * 128],
                        ph1[:, jc * 128:(jc + 1) * 128],
                        mybir.ActivationFunctionType.Silu,
                        bias=b1_sb[:, jc:jc + 1])
            return h1_sb

        def emit_mlp_tail(w, h1_sb):
            ph2 = pm.tile([128, H2], dt.float32, tag="pmlp")
            for kc in range(KC):
                for hc in range(HC):
                    nc.tensor.matmul(
                        ph2[:, kc * 128:(kc + 1) * 128],
                        w2_sb[:, hc, kc * 128:(kc + 1) * 128],
                        h1_sb[:, hc * 128:(hc + 1) * 128],
                        start=(hc == 0), stop=(hc == HC - 1))
            h2_sb = spool.tile([128, H2], m_dt, tag="h2")
            if SILU_COMPOSE:
                z2 = spool.tile([128, H2], dt.float32, tag="z2")
                for kc in range(KC):
                    nc.scalar.activation(
                        z2[:, kc * 128:(kc + 1) * 128],
                        ph2[:, kc * 128:(kc + 1) * 128],
                        mybir.ActivationFunctionType.Identity,
                        bias=b2_sb[:, kc:kc + 1])
                nc.scalar.activation(
                    h2_sb[:], z2[:], mybir.ActivationFunctionType.Sigmoid)
                nc.vector.tensor_mul(h2_sb[:], h2_sb[:], z2[:])
            else:
                for kc in range(KC):
                    nc.scalar.activation(
                        h2_sb[:, kc * 128:(kc + 1) * 128],
                        ph2[:, kc * 128:(kc + 1) * 128],
                        mybir.ActivationFunctionType.Silu,
                        bias=b2_sb[:, kc:kc + 1])
            po = pm.tile([1, SEG_W], dt.float32, tag="pmlp")
            for kc in range(KC):
                nc.tensor.matmul(
                    po[:], w3_sb[:, kc:kc + 1],
                    h2_sb[:, kc * 128:(kc + 1) * 128],
                    start=(kc == 0), stop=(kc == KC - 1))
            nc.scalar.activation(
                out_sb[0:1, w * SEG_W:(w + 1) * SEG_W], po[:],
                mybir.ActivationFunctionType.Identity,
                bias=b3_sb[0:1, 0:1])

        emit_oh_dma(0)
        emit_oh_dma(1)
        emit_gb(0)

        # Software pipeline: window w's MLP is interleaved into window w+1's
        # DR stream so the PE never waits on the evict->transpose->FiLM
        # chain or the silu round-trips.
        film = {}       # w -> gmodT tile
        h1s = {}        # w -> h1 tile
        for w in range(WINDOWS):
            pg_t = pg.tile([128, H], dt.float32)
            emit_dr_half(w, 0, pg_t)
            if w >= 1:
                h1s[w - 1] = emit_mlp_l1(w - 1, film.pop(w - 1))
            emit_dr_half(w, 1, pg_t)
            # evict g early: ACT runs it while the PE does the w-1 MLP tail
            g_sb = spool.tile([128, H], dt.float32, tag="g")
            nc.scalar.copy(g_sb[:], pg_t[:])
            if w >= 1:
                emit_mlp_tail(w - 1, h1s.pop(w - 1))
            emit_gb(w + 1)

            # transpose + FiLM for this window
            g_sbt, b_sbt = gb_tiles.pop(w)
            pt_t = pt.tile([128, H], dt.float32)
            for hc in range(HC):
                nc.tensor.transpose(
                    pt_t[:, hc * 128:(hc + 1) * 128],
                    g_sb[:, hc * 128:(hc + 1) * 128],
                    iden_sb[:])
            gmodT = spool.tile([128, H], m_dt, tag="gmodT")
            pt_v = pt_t[:].rearrange("p (c s) -> p c s", c=HC)
            gm_v = gmodT[:].rearrange("p (c s) -> p c s", c=HC)
            nc.vector.tensor_mul(gm_v, pt_v, g_sbt[:, :, :])
            nc.vector.tensor_add(gm_v, gm_v, b_sbt[:, :, :])
            film[w] = gmodT

            emit_oh_dma(w + 2)

        h1s[WINDOWS - 1] = emit_mlp_l1(WINDOWS - 1, film.pop(WINDOWS - 1))
        emit_mlp_tail(WINDOWS - 1, h1s.pop(WINDOWS - 1))

        nc.sync.dma_start(out, out_sb[:])

    nc.compile()
    return nc


def diffuse_quantize(x: np.ndarray, counts: np.ndarray, starts: np.ndarray):
    """Error-diffusion quantization of x to E4M3, sequential within each
    segment (vectorized over segments x features).  Returns the quantized
    bytes for every node plus, for odd-length segments, a pad value that
    absorbs the final carry."""
    B = len(counts)
    nH = x.shape[1]
    qx = np.empty(x.shape, dtype=E4M3)
    carry = np.zeros((B, nH), np.float32)
    maxn = int(counts.max()) if B else 0
    for k in range(maxn):
        active = np.nonzero(counts > k)[0]
        if len(active) == 0:
            break
        idx = starts[active] + k
        v = x[idx] + carry[active]
        q = np.clip(v, -E4M3_MAX, E4M3_MAX).astype(E4M3)
        qx[idx] = q
        carry[active] = v - q.astype(np.float32)
    odd = np.nonzero((counts % 2 == 1) & (counts > 0))[0]
    pad_q = np.zeros((B, nH), dtype=E4M3)
    if len(odd):
        pad_q[odd] = np.clip(carry[odd], -E4M3_MAX, E4M3_MAX).astype(E4M3)
    return qx, pad_q


def prepare_core_inputs(
    x, batch, domain_emb, gamma_w, gamma_b, beta_w, beta_b,
    w1, b1, w2, b2, w3, b3,
    cp2: int, n_cores: int,
):
    """Quantize, pad, pack and transpose the full inputs into one in_map
    per core."""
    spc = B_SEGS // n_cores
    Cp = 2 * cp2
    cap_pairs = 128 * Cp

    batch = np.ascontiguousarray(np.asarray(batch).astype(np.int64))
    x = np.asarray(x, dtype=np.float32)
    n = x.shape[0]

    counts = np.bincount(batch, minlength=B_SEGS)
    starts = np.concatenate([[0], np.cumsum(counts)])[:B_SEGS]

    qx, pad_q = diffuse_quantize(x, counts, starts)

    # --- build the padded per-segment stream (pairs stay within-segment
    # because every padded run has even length) ---
    odd = (counts % 2).astype(np.int64)
    pads_before = np.concatenate([[0], np.cumsum(odd)])[:B_SEGS]
    pstart = starts + pads_before                       # stream offset per segment
    m_total = int(n + odd.sum())
    pstart_full = np.concatenate([pstart, [m_total]])

    stream = np.zeros((m_total, H), dtype=E4M3)
    node_pos = np.arange(n, dtype=np.int64) + pads_before[batch]
    stream[node_pos] = qx
    stream_seg = np.zeros(m_total, dtype=np.int64)
    stream_seg[node_pos] = batch
    odd_segs = np.nonzero(odd)[0]
    if len(odd_segs):
        pad_pos = pstart[odd_segs] + counts[odd_segs]
        stream[pad_pos] = pad_q[odd_segs]
        stream_seg[pad_pos] = odd_segs

    m_np = BF16
    shared = {
        "gw": np.ascontiguousarray(
            np.concatenate([np.asarray(gamma_w, np.float32).T,
                            np.asarray(gamma_b, np.float32)[None]], axis=0)),
        "bw": np.ascontiguousarray(
            np.concatenate([np.asarray(beta_w, np.float32).T,
                            np.asarray(beta_b, np.float32)[None]], axis=0)),
        "w1t": np.ascontiguousarray(np.asarray(w1, np.float32).T.astype(m_np)),
        "w2t": np.ascontiguousarray(np.asarray(w2, np.float32).T.astype(m_np)),
        "w3c": np.ascontiguousarray(
            np.asarray(w3, np.float32).reshape(H2 // 128, 128).T.astype(m_np)),
        "b1c": np.ascontiguousarray(np.asarray(b1, np.float32).reshape(H // 128, 128).T),
        "b2c": np.ascontiguousarray(np.asarray(b2, np.float32).reshape(H2 // 128, 128).T),
        "b3c": np.asarray(b3, np.float32).reshape(1, 1),
        "iden": np.eye(128, dtype=np.float32),
        "iotr": np.tile(np.arange(128, dtype=np.float32), (128, 1)),
    }

    dom = np.asarray(domain_emb, np.float32)

    in_maps = []
    for core in range(n_cores):
        seg0 = core * spc
        xp_c = np.zeros((WINDOWS, 2, 128, cp2, 2, H), dtype=E4M3)
        brt_c = np.full((128, WINDOWS, 2, cp2), -1.0e9, dtype=np.float32)
        for w in range(WINDOWS):
            s_lo = seg0 + w * SEG_W
            lo = int(pstart_full[s_lo])
            hi = int(pstart_full[s_lo + SEG_W])
            n_pairs = (hi - lo) // 2
            if n_pairs == 0:
                continue
            if n_pairs > cap_pairs:
                raise ValueError(f"window overflow: {n_pairs} > {cap_pairs}")
            sl = stream[lo:hi].reshape(n_pairs, 2, H)
            seg_rel = (stream_seg[lo:hi:2] - s_lo).astype(np.float32)
            # pair i -> partition i%128, column i//128
            arr = np.zeros((cap_pairs, 2, H), dtype=E4M3)
            arr[:n_pairs] = sl
            bflat = np.full(cap_pairs, -1.0e9, dtype=np.float32)
            bflat[:n_pairs] = seg_rel
            # [Cp, 128, 2, H] -> [jhalf, 128, cp2, 2, H]
            a5 = arr.reshape(2, cp2, 128, 2, H).transpose(0, 2, 1, 3, 4)
            xp_c[w] = a5
            brt_c[:, w] = bflat.reshape(2, cp2, 128).transpose(2, 0, 1)
        dombT_c = np.ascontiguousarray(
            np.concatenate([dom[seg0:seg0 + spc].T,
                            np.ones((1, spc), np.float32)], axis=0))
        in_maps.append({
            "xp": np.ascontiguousarray(xp_c),
            "brtA": np.ascontiguousarray(brt_c),
            "dombT": dombT_c, **shared})
    return in_maps


def _pick_cp2(batch: np.ndarray, n_cores: int) -> int:
    """Half-window pair-column count: max padded pair count over all
    128-segment windows, in units of 128 pairs, rounded up to even."""
    counts = np.bincount(batch, minlength=B_SEGS)
    odd = (counts % 2).astype(np.int64)
    starts = np.concatenate([[0], np.cumsum(counts)])
    pads_before = np.concatenate([[0], np.cumsum(odd)])
    pstart = starts + pads_before                      # [B+1]
    edges = pstart[::SEG_W]                            # window boundaries
    pairs = np.diff(edges) // 2
    cp = max(1, int(np.max(pairs) + 127) // 128)
    return (cp + 1) // 2


_PROGRAM_CACHE: dict = {}

# Set by test harnesses: request an NTFF trace and stash the raw results.
TRACE = False
LAST_RESULT = None


def kernel(**inputs) -> np.ndarray:
    x = np.asarray(inputs["x"], dtype=np.float32)
    batch = np.ascontiguousarray(np.asarray(inputs["batch"]).astype(np.int64))
    assert x.shape == (N_NODES, H), x.shape

    cp2 = _pick_cp2(batch, N_CORES)

    key = (cp2, N_CORES)
    if key not in _PROGRAM_CACHE:
        _PROGRAM_CACHE[key] = build_program(cp2, N_CORES)
    nc = _PROGRAM_CACHE[key]

    in_maps = prepare_core_inputs(
        x, batch,
        inputs["domain_emb"], inputs["gamma_w"], inputs["gamma_b"],
        inputs["beta_w"], inputs["beta_b"],
        inputs["w1"], inputs["b1"], inputs["w2"], inputs["b2"],
        inputs["w3"], inputs["b3"],
        cp2, N_CORES,
    )

    res = bass_utils.run_bass_kernel_spmd(
        nc, in_maps, core_ids=list(range(N_CORES)), trace=TRACE)
    global LAST_RESULT
    LAST_RESULT = res
    out = np.concatenate([res.results[c]["out"].reshape(-1) for c in range(N_CORES)])
    return np.ascontiguousarray(out.astype(np.float32))
